# revision 4
# baseline (speedup 1.0000x reference)
"""BinnedColorLoss Trainium2 kernel (v3).

loss = -mean_{b,h,w}[ (sum_k logp[b, idx_k, h, w] * wts_k) * w ]
with logp = log_softmax(pred, axis=1), idx/wts/w gathered per-pixel from
313-entry KNN tables via the pixel's bin t = binned_color[b,0,h,w].

Math restructuring (per pixel, t = bin, lse = logsumexp over C):
  sum_k logp[idx_k]*wts_k*w = (sum_k pred[idx_k]*wts_k)*w - lse * (w*sum_k wts_k)
With A[t,c] = w[t]*sum_k wts[t,k]*[idx[t,k]=c] and coef(pix) = w[t]*sum_k wts,
and N = B*H*W:
  loss = ( sum_pix lse(pix)*coef(pix) - sum_pix <A[t(pix)], pred[:,pix]> ) / N

Device strategy (v3; data-parallel over 8 cores, 2 images each):
  - Pixels are SORTED BY BIN on the host (the loss is an order-free mean).
    Each 2048-pixel block then spans <=22 consecutive bins, so the G-term
    becomes: per 16-chunk block, one-hot "segment" DoubleRow matmuls
    S[j, c] += onehot[pix, j] * pred[pix, c]  (j = bin - t0, 64-wide window)
    accumulated in PSUM over 8 chunk-pairs, then ONE scalar_tensor_tensor
    dot against the window's A-rows (streamed from host, 64x320 fp8/blk).
    This removes the 10.5MB dense A-row stream of v2 entirely.
  - lse path: chunks are split between ACT (true exp, fp8 in f16 out) and
    GPSIMD (Schraudolph fast-exp: one tensor_scalar fp8->i16 computing
    round(1024*(x*log2e + 15) + C); the i16 bit pattern IS ~exp(x) when
    bitcast to f16). Both write the same f16 tile; one DVE halving add
    tree per group sums 320 channels -> sume; Ln at piece boundaries.
  - DMA: pred (10.5MB) + onehot B (2.1MB) + A windows (0.33MB) + coef,
    pred transfers alternate between the Sync and Tensor DGE rings.
Host combines the 8 per-core [128, 8] partials: loss = (L - G)/N.
"""

import os
import sys

for _p in ("/opt/trn_rl_repo",):
    if _p not in sys.path:
        sys.path.insert(0, _p)

from contextlib import ExitStack

import numpy as np

import concourse.bacc as bacc
import concourse.bass as bass  # noqa: F401
import concourse.mybir as mybir
from concourse import bass_utils, tile

F32 = mybir.dt.float32
F16 = mybir.dt.float16
I16 = mybir.dt.int16
FP8 = mybir.dt.float8e4

B, C, H, W, K = 16, 313, 128, 128, 5
CP = 320                   # C padded (even tree levels, aligned rows)
NCORES = 8
BPC = B // NCORES          # images per core
PIX = BPC * H * W          # pixels per core (32768)
P = 128                    # pixels per chunk (partition dim)
NCHUNK = PIX // P          # 256
G = 16                     # max chunks per group (tile size)
NTOT = B * H * W           # mean denominator
WIN = 64                   # bin window per 16-chunk block (span <= ~22)
NBLK = NCHUNK // G         # 16 PSUM blocks
PAD_VAL = -10.0            # pad: exp()~4.5e-5, fastexp bits ~527 -> ~3e-5
# Schraudolph fast-exp constants: i16 = round(x*SCHS + SCHB); bitcast f16.
LOG2E = 1.4426950408889634
SCHS = 1024.0 * LOG2E
SCHB = 1024.0 * 15 - 60.0
# warm-up/cool-down schedule: small first groups so the first ACT starts
# early, small last groups so the final tree+Ln tail is short
G_LIST = [4, 4, 8] + [16] * 14 + [8, 4, 4]   # chunks per group (sums to 256)
# lse pieces: (first_grp, end_grp, n_chunks, emit_after_grp, out_col).
PIECES = [
    (0, 10, 128, 13, 5),     # chunks [0,128)   ready g9,  Ln after grp 13
    (10, 19, 124, 19, 6),    # chunks [128,252) ready g18, Ln after grp 19
    (19, 20, 4, -1, 0),      # chunks [252,256) final tail (last 4 chunks)
]


def build_program():
    act_frac = float(os.environ.get("KERNEL_ACT_FRAC", "0.6875"))
    stt_eng = os.environ.get("KERNEL_STT_ENGINE", "vector")
    red_eng = os.environ.get("KERNEL_RED_ENGINE", "vector")
    dma_rings = int(os.environ.get("KERNEL_DMA_RINGS", "2"))

    nc = bacc.Bacc(
        "TRN2",
        target_bir_lowering=False,
        debug=False,
        enable_asserts=False,
        num_devices=NCORES,
    )
    # Prefer an activation-table set containing BOTH Exp and Ln so the
    # mid-run Ln pieces don't force ~1.3us exp<->ln table re-loads.
    if os.environ.get("KERNEL_TABLE_REORDER", "1") == "1":
        import concourse.hw_specs as hw_specs

        tabs = hw_specs.get_activation_tables(nc.m.arch)
        _E = mybir.ActivationFunctionType.Exp
        _L = mybir.ActivationFunctionType.Ln
        if any(_E in v and _L in v for v in tabs.values()):
            combined = next(k for k, v in tabs.items() if _E in v and _L in v)
            for k, v in tabs.items():
                if k != combined:
                    v.discard(_E)
                    v.discard(_L)

    pred_d = nc.dram_tensor("pred_t", [P, NCHUNK, CP], FP8, kind="ExternalInput").ap()
    bmat_d = nc.dram_tensor("bmat_t", [P, NCHUNK, WIN], FP8, kind="ExternalInput").ap()
    awin_d = nc.dram_tensor("awin_t", [WIN, NBLK, CP], FP8, kind="ExternalInput").ap()
    coef_d = nc.dram_tensor("coef_t", [P, NCHUNK], F32, kind="ExternalInput").ap()
    out_d = nc.dram_tensor("out", [P, 8], F32, kind="ExternalOutput").ap()

    with tile.TileContext(nc) as tc, ExitStack() as ctx, nc.allow_low_precision(
        "f16 exp-sum tree + fp8 G matmuls; validated rel err ~1e-3 << 2e-2 tol"
    ):
        const = ctx.enter_context(tc.tile_pool(name="const", bufs=1))
        predp = ctx.enter_context(tc.tile_pool(name="pred", bufs=4))
        bp = ctx.enter_context(tc.tile_pool(name="bm", bufs=4))
        expp = ctx.enter_context(tc.tile_pool(name="exp", bufs=3))
        trp = ctx.enter_context(tc.tile_pool(name="tree", bufs=3))
        accp = ctx.enter_context(tc.tile_pool(name="acc", bufs=1))
        psum = ctx.enter_context(tc.tile_pool(name="psum", bufs=2, space="PSUM"))

        ngrp = len(G_LIST)
        starts = [sum(G_LIST[:i]) for i in range(ngrp)]

        stt_q = getattr(nc, stt_eng)
        red_q = getattr(nc, red_eng)

        # DMA issue: pred+B per group, two groups ahead of the consumer.
        # pred transfers alternate Sync/Tensor rings; B on Sync.
        tiles = {}

        def issue_grp(g):
            if g >= ngrp or g in tiles:
                return
            c0, gsz = starts[g], G_LIST[g]
            pt = predp.tile([P, G, CP], FP8, tag="pred", name=f"pred{g}")
            q = nc.sync if (dma_rings == 1 or g % 2 == 0) else nc.gpsimd
            q.dma_start(pt[:, 0:gsz, :], pred_d[:, c0:c0 + gsz, :])
            bt = bp.tile([P, G, WIN], FP8, tag="bm", name=f"bm{g}")
            nc.sync.dma_start(bt[:, 0:gsz, :], bmat_d[:, c0:c0 + gsz, :])
            tiles[g] = (pt, bt)

        issue_grp(0)
        issue_grp(1)

        coef_t = const.tile([P, NCHUNK], F32, tag="coef")
        nc.sync.dma_start(coef_t[:], coef_d)
        awin_t = const.tile([WIN, NBLK, CP], FP8, tag="awin")
        nc.sync.dma_start(awin_t[:], awin_d)

        out_t = accp.tile([P, 8], F32, tag="out")
        nc.vector.memset(out_t[:], 0.0)
        gwin_t = accp.tile([WIN, NBLK], F32, tag="gwin")
        sdot_t = accp.tile([WIN, CP], F16, tag="sdot")

        # per-piece lse state
        grp_piece = {}
        psume, plse, pscr = [], [], []
        for pi, (g_lo, g_hi, nch, _, _) in enumerate(PIECES):
            psume.append(accp.tile([P, nch], F32, tag=f"sume{pi}", name=f"sume{pi}"))
            plse.append(accp.tile([P, nch], F32, tag=f"lse{pi}", name=f"lse{pi}"))
            pscr.append(accp.tile([P, nch], F32, tag=f"pscr{pi}", name=f"pscr{pi}"))
            for g in range(g_lo, g_hi):
                grp_piece[g] = pi

        def emit_lse_piece(pi):
            g_lo, g_hi, nch, _, col = PIECES[pi]
            p_lo = starts[g_lo]
            nc.scalar.activation(
                plse[pi][:], psume[pi][:], mybir.ActivationFunctionType.Ln
            )
            nc.vector.tensor_mul(
                pscr[pi][:], plse[pi][:], coef_t[:, p_lo:p_lo + nch]
            )
            nc.vector.tensor_reduce(
                out_t[:, col:col + 1],
                pscr[pi][:],
                axis=mybir.AxisListType.X,
                op=mybir.AluOpType.add,
            )

        s_ps = [None] * NBLK

        def emit_pair(pair):
            """G-term matmul for chunks (2*pair, 2*pair+1)."""
            blk = (2 * pair) // G
            g = next(gg for gg in range(ngrp)
                     if starts[gg] <= 2 * pair < starts[gg] + G_LIST[gg])
            pt, bt = tiles[g]
            o = 2 * pair - starts[g]
            first = (2 * pair) % G == 0
            last = (2 * pair + 2) % G == 0
            if first:
                s_ps[blk] = psum.tile([WIN, CP], F32, tag="s", name=f"s{blk}")
            nc.tensor.matmul(
                s_ps[blk][:, :],
                bt[:, o:o + 2, :],
                pt[:, o:o + 2, :],
                start=first,
                stop=last,
                perf_mode=mybir.MatmulPerfMode.DoubleRow,
            )
            if last:
                stt_q.scalar_tensor_tensor(
                    sdot_t[:],
                    s_ps[blk][:, :],
                    1.0,
                    awin_t[:, blk, :],
                    mybir.AluOpType.mult,
                    mybir.AluOpType.mult,
                    accum_out=gwin_t[:, blk:blk + 1],
                )

        for g in range(ngrp):
            c0, gsz = starts[g], G_LIST[g]
            issue_grp(g + 2)
            pt, bt = tiles[g]
            pi = grp_piece[g]
            s_lo = c0 - starts[PIECES[pi][0]]
            a_g = max(1, int(round(act_frac * gsz)))

            # lse path: ACT exp for chunks [0:a_g], GPSIMD fastexp for the
            # rest -- both land in the same f16 tile (fastexp via i16 bitcast)
            et = expp.tile([P, G, CP], F16, tag="exp")
            nc.scalar.activation(
                et[:, 0:a_g, :], pt[:, 0:a_g, :],
                mybir.ActivationFunctionType.Exp,
            )
            if a_g < gsz:
                nc.gpsimd.tensor_scalar(
                    et[:, a_g:gsz, :].bitcast(I16),
                    pt[:, a_g:gsz, :],
                    SCHS,
                    SCHB,
                    mybir.AluOpType.mult,
                    mybir.AluOpType.add,
                )
            tr = trp.tile([P, G, 300], F16, tag="tree")
            nc.vector.tensor_add(
                tr[:, 0:gsz, 0:160], et[:, 0:gsz, 0:160], et[:, 0:gsz, 160:320]
            )
            nc.vector.tensor_add(
                tr[:, 0:gsz, 160:240], tr[:, 0:gsz, 0:80], tr[:, 0:gsz, 80:160]
            )
            nc.vector.tensor_add(
                tr[:, 0:gsz, 240:280], tr[:, 0:gsz, 160:200], tr[:, 0:gsz, 200:240]
            )
            nc.vector.tensor_add(
                tr[:, 0:gsz, 280:300], tr[:, 0:gsz, 240:260], tr[:, 0:gsz, 260:280]
            )
            red_q.tensor_reduce(
                psume[pi][:, s_lo:s_lo + gsz],
                tr[:, 0:gsz, 280:300],
                axis=mybir.AxisListType.X,
                op=mybir.AluOpType.add,
            )
            for pj, (_, _, _, emit_after, _) in enumerate(PIECES):
                if emit_after == g:
                    emit_lse_piece(pj)

            # G path: all chunk-pairs of this group
            for pr in range(c0 // 2, (c0 + gsz) // 2):
                emit_pair(pr)

            tiles.pop(g)

        # final tail: last lse piece; G window-dot partials
        emit_lse_piece(len(PIECES) - 1)
        nc.vector.tensor_reduce(
            out_t[0:WIN, 1:2],
            gwin_t[:],
            axis=mybir.AxisListType.X,
            op=mybir.AluOpType.add,
        )
        nc.sync.dma_start(out_d, out_t[:])

    nc.compile()
    return nc


def host_inputs(pred, binned_color, knn_idx, knn_weights, weights):
    """Per-core input dicts. pred (B,C,H,W) f32; binned (B,1,H,W) int;
    knn_idx (C,K) int; knn_weights (C,K) f32; weights (C,) f32."""
    import ml_dtypes

    fp8 = ml_dtypes.float8_e4m3

    pred = np.asarray(pred, dtype=np.float32)
    binned = np.asarray(binned_color)
    knn_idx = np.asarray(knn_idx).astype(np.int64)
    knn_w = np.asarray(knn_weights, dtype=np.float32)
    wts = np.asarray(weights, dtype=np.float32)

    # A[t, c] = w[t] * sum_k knn_w[t,k] * [knn_idx[t,k] == c], padded to CP
    a_tab = np.zeros((C, CP), dtype=np.float32)
    rows = np.repeat(np.arange(C), K)
    cols = knn_idx.reshape(-1)
    vals = (wts[:, None] * knn_w).reshape(-1)
    np.add.at(a_tab, (rows, cols), vals)
    a_tab8 = a_tab.astype(fp8)

    coef_full = wts * knn_w.sum(axis=1)          # (C,)

    in_maps = []
    for core in range(NCORES):
        bs = slice(core * BPC, (core + 1) * BPC)
        t = binned[bs, 0].reshape(PIX).astype(np.int64)
        order = np.argsort(t, kind="stable")
        ts = t[order]

        pm = np.full((PIX, CP), PAD_VAL, dtype=np.float32)
        pm[:, :C] = pred[bs].transpose(0, 2, 3, 1).reshape(PIX, C)[order]
        pred_t = np.ascontiguousarray(
            pm.reshape(NCHUNK, P, CP).transpose(1, 0, 2)
        ).astype(fp8)

        # per-block bin windows + one-hot B
        t0 = np.repeat(ts.reshape(NBLK, G * P)[:, 0], G * P)
        j = ts - t0
        assert j.min() >= 0 and j.max() < WIN, f"window overflow: {j.max()}"
        bm = np.zeros((PIX, WIN), dtype=fp8)
        bm[np.arange(PIX), j] = fp8(1.0)
        bmat = np.ascontiguousarray(
            bm.reshape(NCHUNK, P, WIN).transpose(1, 0, 2)
        )

        # A window rows: awin[j, blk, :] = A[t0_blk + j]
        t0_blk = ts.reshape(NBLK, G * P)[:, 0]          # (NBLK,)
        idx = t0_blk[None, :] + np.arange(WIN)[:, None]  # (WIN, NBLK)
        awin = np.zeros((WIN, NBLK, CP), dtype=fp8)
        ok = idx < C
        awin[ok] = a_tab8[idx[ok]]

        coef = np.ascontiguousarray(
            coef_full[ts].reshape(NCHUNK, P).T
        ).astype(np.float32)

        in_maps.append(
            {
                "pred_t": pred_t,
                "bmat_t": bmat,
                "awin_t": awin,
                "coef_t": coef,
            }
        )
    return in_maps


def combine_outputs(core_outs):
    """core_outs: list of [128, 8] f32 arrays -> scalar loss."""
    total = 0.0
    for o in core_outs:
        o = o.astype(np.float64)
        lsec = o[:, 0].sum() + o[:, 5].sum() + o[:, 6].sum()
        g = o[0:WIN, 1].sum()
        total += lsec - g
    return np.array(total / NTOT, dtype=np.float32)


_NC_CACHE = None


def kernel(pred, _color, binned_color, knn_idx, knn_weights, weights):
    global _NC_CACHE
    if _NC_CACHE is None:
        _NC_CACHE = build_program()
    nc = _NC_CACHE
    in_maps = host_inputs(pred, binned_color, knn_idx, knn_weights, weights)
    res = bass_utils.run_bass_kernel_spmd(nc, in_maps, core_ids=list(range(NCORES)))
    outs = [res.results[i]["out"] for i in range(NCORES)]
    return combine_outputs(outs)


if __name__ == "__main__":
    import jax
    import reference

    with jax.default_device(jax.devices("cpu")[0]):
        inputs = reference.setup_inputs()
        inputs = {k: np.asarray(jax.device_get(v)) for k, v in inputs.items()}
    got = kernel(**inputs)
    print("kernel loss:", got)


# revision 7
# speedup vs baseline: 1.0527x; 1.0527x over previous
"""BinnedColorLoss Trainium2 kernel (v3).

loss = -mean_{b,h,w}[ (sum_k logp[b, idx_k, h, w] * wts_k) * w ]
with logp = log_softmax(pred, axis=1), idx/wts/w gathered per-pixel from
313-entry KNN tables via the pixel's bin t = binned_color[b,0,h,w].

Math restructuring (per pixel, t = bin, lse = logsumexp over C):
  sum_k logp[idx_k]*wts_k*w = (sum_k pred[idx_k]*wts_k)*w - lse * (w*sum_k wts_k)
With A[t,c] = w[t]*sum_k wts[t,k]*[idx[t,k]=c] and coef(pix) = w[t]*sum_k wts,
and N = B*H*W:
  loss = ( sum_pix lse(pix)*coef(pix) - sum_pix <A[t(pix)], pred[:,pix]> ) / N

Device strategy (v3; data-parallel over 8 cores, 2 images each):
  - Pixels are SORTED BY BIN on the host (the loss is an order-free mean).
    Each 2048-pixel block then spans <=22 consecutive bins, so the G-term
    becomes: per 16-chunk block, one-hot "segment" DoubleRow matmuls
    S[j, c] += onehot[pix, j] * pred[pix, c]  (j = bin - t0, 64-wide window)
    accumulated in PSUM over 8 chunk-pairs, then ONE scalar_tensor_tensor
    dot against the window's A-rows (streamed from host, 64x320 fp8/blk).
    This removes the 10.5MB dense A-row stream of v2 entirely.
  - lse path: chunks are split between ACT (true exp, fp8 in f16 out) and
    GPSIMD (Schraudolph fast-exp: one tensor_scalar fp8->i16 computing
    round(1024*(x*log2e + 15) + C); the i16 bit pattern IS ~exp(x) when
    bitcast to f16). Both write the same f16 tile; one DVE halving add
    tree per group sums 320 channels -> sume; Ln at piece boundaries.
  - DMA: pred (10.5MB) + onehot B (2.1MB) + A windows (0.33MB) + coef,
    pred transfers alternate between the Sync and Tensor DGE rings.
Host combines the 8 per-core [128, 8] partials: loss = (L - G)/N.
"""

import os
import sys

for _p in ("/opt/trn_rl_repo",):
    if _p not in sys.path:
        sys.path.insert(0, _p)

from contextlib import ExitStack

import numpy as np

import concourse.bacc as bacc
import concourse.bass as bass  # noqa: F401
import concourse.mybir as mybir
from concourse import bass_utils, tile

F32 = mybir.dt.float32
F16 = mybir.dt.float16
I16 = mybir.dt.int16
FP8 = mybir.dt.float8e4

B, C, H, W, K = 16, 313, 128, 128, 5
CP = 320                   # C padded (even tree levels, aligned rows)
NCORES = 8
BPC = B // NCORES          # images per core
PIX = BPC * H * W          # pixels per core (32768)
P = 128                    # pixels per chunk (partition dim)
NCHUNK = PIX // P          # 256
G = 16                     # max chunks per group (tile size)
NTOT = B * H * W           # mean denominator
WIN = 64                   # bin window per 16-chunk block (span <= ~22)
NBLK = NCHUNK // G         # 16 PSUM blocks
PAD_VAL = -10.0            # pad: exp()~4.5e-5, fastexp bits ~527 -> ~3e-5
# Schraudolph fast-exp constants: i16 = round(x*SCHS + SCHB); bitcast f16.
LOG2E = 1.4426950408889634
SCHS = 1024.0 * LOG2E
SCHB = 1024.0 * 15 - 60.0
# warm-up/cool-down schedule: small first groups so the first ACT starts
# early, small last groups so the final tree+Ln tail is short
G_LIST = [4, 4, 8] + [16] * 14 + [8, 4, 4]   # chunks per group (sums to 256)
# lse pieces: (first_grp, end_grp, n_chunks, emit_after_grp, out_col).
PIECES = [
    (0, 10, 128, 13, 5),     # chunks [0,128)   ready g9,  Ln after grp 13
    (10, 19, 124, 19, 6),    # chunks [128,252) ready g18, Ln after grp 19
    (19, 20, 4, -1, 0),      # chunks [252,256) final tail (last 4 chunks)
]


def build_program():
    act_frac = float(os.environ.get("KERNEL_ACT_FRAC", "0.67"))
    stt_eng = os.environ.get("KERNEL_STT_ENGINE", "vector")
    gps_red = {
        int(x) for x in os.environ.get("KERNEL_GPSRED", "6,10,14").split(",") if x
    }
    dma_rings = int(os.environ.get("KERNEL_DMA_RINGS", "2"))

    nc = bacc.Bacc(
        "TRN2",
        target_bir_lowering=False,
        debug=False,
        enable_asserts=False,
        num_devices=NCORES,
    )
    # Prefer an activation-table set containing BOTH Exp and Ln so the
    # mid-run Ln pieces don't force ~1.3us exp<->ln table re-loads.
    if os.environ.get("KERNEL_TABLE_REORDER", "1") == "1":
        import concourse.hw_specs as hw_specs

        tabs = hw_specs.get_activation_tables(nc.m.arch)
        _E = mybir.ActivationFunctionType.Exp
        _L = mybir.ActivationFunctionType.Ln
        if any(_E in v and _L in v for v in tabs.values()):
            combined = next(k for k, v in tabs.items() if _E in v and _L in v)
            for k, v in tabs.items():
                if k != combined:
                    v.discard(_E)
                    v.discard(_L)

    pab_d = nc.dram_tensor(
        "pab_t", [P, NCHUNK, CP + WIN], FP8, kind="ExternalInput"
    ).ap()
    awin_d = nc.dram_tensor("awin_t", [WIN, NBLK, CP], FP8, kind="ExternalInput").ap()
    coef_d = nc.dram_tensor("coef_t", [P, NCHUNK], F32, kind="ExternalInput").ap()
    out_d = nc.dram_tensor("out", [P, 8], F32, kind="ExternalOutput").ap()

    with tile.TileContext(nc) as tc, ExitStack() as ctx, nc.allow_low_precision(
        "f16 exp-sum tree + fp8 G matmuls; validated rel err ~1e-3 << 2e-2 tol"
    ):
        const = ctx.enter_context(tc.tile_pool(name="const", bufs=1))
        predp = ctx.enter_context(tc.tile_pool(name="pred", bufs=4))
        expp = ctx.enter_context(tc.tile_pool(name="exp", bufs=3))
        trp = ctx.enter_context(tc.tile_pool(name="tree", bufs=3))
        accp = ctx.enter_context(tc.tile_pool(name="acc", bufs=1))
        psum = ctx.enter_context(tc.tile_pool(name="psum", bufs=2, space="PSUM"))

        ngrp = len(G_LIST)
        starts = [sum(G_LIST[:i]) for i in range(ngrp)]

        stt_q = getattr(nc, stt_eng)

        # DMA issue: pred+B per group, two groups ahead of the consumer.
        # pred transfers alternate Sync/Tensor rings; B on Sync.
        tiles = {}

        def issue_grp(g):
            if g >= ngrp or g in tiles:
                return
            c0, gsz = starts[g], G_LIST[g]
            pt = predp.tile([P, G, CP + WIN], FP8, tag="pred", name=f"pred{g}")
            q = nc.sync if (dma_rings == 1 or g % 2 == 0) else nc.gpsimd
            q.dma_start(pt[:, 0:gsz, :], pab_d[:, c0:c0 + gsz, :])
            tiles[g] = pt

        issue_grp(0)
        issue_grp(1)

        coef_t = const.tile([P, NCHUNK], F32, tag="coef")
        nc.sync.dma_start(coef_t[:], coef_d)
        awin_t = const.tile([WIN, NBLK, CP], FP8, tag="awin")
        nc.sync.dma_start(awin_t[:], awin_d)

        out_t = accp.tile([P, 8], F32, tag="out")
        nc.vector.memset(out_t[:], 0.0)
        gwin_t = accp.tile([WIN, NBLK], F32, tag="gwin")
        sdot_t = accp.tile([WIN, CP], F16, tag="sdot")

        # per-piece lse state
        grp_piece = {}
        psume, plse, pscr = [], [], []
        for pi, (g_lo, g_hi, nch, _, _) in enumerate(PIECES):
            psume.append(accp.tile([P, nch], F32, tag=f"sume{pi}", name=f"sume{pi}"))
            plse.append(accp.tile([P, nch], F32, tag=f"lse{pi}", name=f"lse{pi}"))
            pscr.append(accp.tile([P, nch], F32, tag=f"pscr{pi}", name=f"pscr{pi}"))
            for g in range(g_lo, g_hi):
                grp_piece[g] = pi

        def emit_lse_piece(pi):
            g_lo, g_hi, nch, _, col = PIECES[pi]
            p_lo = starts[g_lo]
            nc.scalar.activation(
                plse[pi][:], psume[pi][:], mybir.ActivationFunctionType.Ln
            )
            nc.vector.tensor_mul(
                pscr[pi][:], plse[pi][:], coef_t[:, p_lo:p_lo + nch]
            )
            nc.vector.tensor_reduce(
                out_t[:, col:col + 1],
                pscr[pi][:],
                axis=mybir.AxisListType.X,
                op=mybir.AluOpType.add,
            )

        s_ps = [None] * NBLK

        def emit_pair(pair):
            """G-term matmul for chunks (2*pair, 2*pair+1)."""
            blk = (2 * pair) // G
            g = next(gg for gg in range(ngrp)
                     if starts[gg] <= 2 * pair < starts[gg] + G_LIST[gg])
            pt = tiles[g]
            o = 2 * pair - starts[g]
            first = (2 * pair) % G == 0
            last = (2 * pair + 2) % G == 0
            if first:
                s_ps[blk] = psum.tile([WIN, CP], F32, tag="s", name=f"s{blk}")
            nc.tensor.matmul(
                s_ps[blk][:, :],
                pt[:, o:o + 2, CP:CP + WIN],
                pt[:, o:o + 2, 0:CP],
                start=first,
                stop=last,
                perf_mode=mybir.MatmulPerfMode.DoubleRow,
            )
            if last:
                stt_q.scalar_tensor_tensor(
                    sdot_t[:],
                    s_ps[blk][:, :],
                    1.0,
                    awin_t[:, blk, :],
                    mybir.AluOpType.mult,
                    mybir.AluOpType.mult,
                    accum_out=gwin_t[:, blk:blk + 1],
                )

        for g in range(ngrp):
            c0, gsz = starts[g], G_LIST[g]
            issue_grp(g + 2)
            pt = tiles[g]
            pi = grp_piece[g]
            s_lo = c0 - starts[PIECES[pi][0]]
            a_g = max(1, int(round(act_frac * gsz)))

            # lse path: ACT exp for chunks [0:a_g], GPSIMD fastexp for the
            # rest -- both land in the same f16 tile (fastexp via i16 bitcast)
            et = expp.tile([P, G, CP], F16, tag="exp")
            nc.scalar.activation(
                et[:, 0:a_g, :], pt[:, 0:a_g, 0:CP],
                mybir.ActivationFunctionType.Exp,
            )
            if a_g < gsz:
                nc.gpsimd.tensor_scalar(
                    et[:, a_g:gsz, :].bitcast(I16),
                    pt[:, a_g:gsz, 0:CP],
                    SCHS,
                    SCHB,
                    mybir.AluOpType.mult,
                    mybir.AluOpType.add,
                )
            tr = trp.tile([P, G, 160], F16, tag="tree")
            nc.vector.tensor_add(
                tr[:, 0:gsz, 0:160], et[:, 0:gsz, 0:160], et[:, 0:gsz, 160:320]
            )
            nc.vector.tensor_add(
                tr[:, 0:gsz, 0:80], tr[:, 0:gsz, 0:80], tr[:, 0:gsz, 80:160]
            )
            nc.vector.tensor_reduce(
                psume[pi][:, s_lo:s_lo + gsz],
                tr[:, 0:gsz, 0:80],
                axis=mybir.AxisListType.X,
                op=mybir.AluOpType.add,
            )
            for pj, (_, _, _, emit_after, _) in enumerate(PIECES):
                if emit_after == g:
                    emit_lse_piece(pj)

            # G path: all chunk-pairs of this group
            for pr in range(c0 // 2, (c0 + gsz) // 2):
                emit_pair(pr)

            tiles.pop(g)

        # final tail: last lse piece; G window-dot partials
        emit_lse_piece(len(PIECES) - 1)
        nc.vector.tensor_reduce(
            out_t[0:WIN, 1:2],
            gwin_t[:],
            axis=mybir.AxisListType.X,
            op=mybir.AluOpType.add,
        )
        nc.sync.dma_start(out_d, out_t[:])

    nc.compile()
    return nc


def host_inputs(pred, binned_color, knn_idx, knn_weights, weights):
    """Per-core input dicts. pred (B,C,H,W) f32; binned (B,1,H,W) int;
    knn_idx (C,K) int; knn_weights (C,K) f32; weights (C,) f32."""
    import ml_dtypes

    fp8 = ml_dtypes.float8_e4m3

    pred = np.asarray(pred, dtype=np.float32)
    binned = np.asarray(binned_color)
    knn_idx = np.asarray(knn_idx).astype(np.int64)
    knn_w = np.asarray(knn_weights, dtype=np.float32)
    wts = np.asarray(weights, dtype=np.float32)

    # A[t, c] = w[t] * sum_k knn_w[t,k] * [knn_idx[t,k] == c], padded to CP
    a_tab = np.zeros((C, CP), dtype=np.float32)
    rows = np.repeat(np.arange(C), K)
    cols = knn_idx.reshape(-1)
    vals = (wts[:, None] * knn_w).reshape(-1)
    np.add.at(a_tab, (rows, cols), vals)
    a_tab8 = a_tab.astype(fp8)

    coef_full = wts * knn_w.sum(axis=1)          # (C,)

    in_maps = []
    for core in range(NCORES):
        bs = slice(core * BPC, (core + 1) * BPC)
        t = binned[bs, 0].reshape(PIX).astype(np.int64)
        order = np.argsort(t, kind="stable")
        ts = t[order]

        pm = np.full((PIX, CP), PAD_VAL, dtype=np.float32)
        pm[:, :C] = pred[bs].transpose(0, 2, 3, 1).reshape(PIX, C)[order]

        # per-block bin windows + one-hot B
        t0 = np.repeat(ts.reshape(NBLK, G * P)[:, 0], G * P)
        j = ts - t0
        assert j.min() >= 0 and j.max() < WIN, f"window overflow: {j.max()}"
        pab = np.zeros((PIX, CP + WIN), dtype=fp8)
        pab[:, 0:CP] = pm.astype(fp8)
        pab[np.arange(PIX), CP + j] = fp8(1.0)
        pab_t = np.ascontiguousarray(
            pab.reshape(NCHUNK, P, CP + WIN).transpose(1, 0, 2)
        )

        # A window rows: awin[j, blk, :] = A[t0_blk + j]
        t0_blk = ts.reshape(NBLK, G * P)[:, 0]          # (NBLK,)
        idx = t0_blk[None, :] + np.arange(WIN)[:, None]  # (WIN, NBLK)
        awin = np.zeros((WIN, NBLK, CP), dtype=fp8)
        ok = idx < C
        awin[ok] = a_tab8[idx[ok]]

        coef = np.ascontiguousarray(
            coef_full[ts].reshape(NCHUNK, P).T
        ).astype(np.float32)

        in_maps.append(
            {
                "pab_t": pab_t,
                "awin_t": awin,
                "coef_t": coef,
            }
        )
    return in_maps


def combine_outputs(core_outs):
    """core_outs: list of [128, 8] f32 arrays -> scalar loss."""
    total = 0.0
    for o in core_outs:
        o = o.astype(np.float64)
        lsec = o[:, 0].sum() + o[:, 5].sum() + o[:, 6].sum()
        g = o[0:WIN, 1].sum()
        total += lsec - g
    return np.array(total / NTOT, dtype=np.float32)


_NC_CACHE = None


def kernel(pred, _color, binned_color, knn_idx, knn_weights, weights):
    global _NC_CACHE
    if _NC_CACHE is None:
        _NC_CACHE = build_program()
    nc = _NC_CACHE
    in_maps = host_inputs(pred, binned_color, knn_idx, knn_weights, weights)
    res = bass_utils.run_bass_kernel_spmd(nc, in_maps, core_ids=list(range(NCORES)))
    outs = [res.results[i]["out"] for i in range(NCORES)]
    return combine_outputs(outs)


if __name__ == "__main__":
    import jax
    import reference

    with jax.default_device(jax.devices("cpu")[0]):
        inputs = reference.setup_inputs()
        inputs = {k: np.asarray(jax.device_get(v)) for k, v in inputs.items()}
    got = kernel(**inputs)
    print("kernel loss:", got)
